# revision 17
# baseline (speedup 1.0000x reference)
"""Trainium2 Bass kernel for nn_BiAttention (RMSNorm + QKV + RoPE + bidirectional
attention + out-proj + residual), tensor-parallel over heads across 8 NeuronCores.

Sharding (Megatron-style): each core owns 2 of 16 heads. It computes the full
RMSNorm statistics and QKV projection for its head slice from the (replicated)
input, runs attention for its heads, and projects through its slice of w_out
rows. The 8 partial outputs are summed on the host, where the residual is added.

Dataflow is fully "transposed": activations live as [feature, token] so every
matmul contracts along the SBUF partition axis with zero on-device transposes
of the big activation (x^T is prepared host-side). Only V is transposed
on-device (PE transpose) to [token, feature] for the attention*V matmul.

softmax skips the max-subtraction: scores are q.k/8 of RMS-normalized,
rotary-projected activations, |s| < ~7 for these inputs, so exp() is safe in
fp32 (verified against the reference in test.py).

All tensors in the matmul dataflow carry MM_DT (float32r = single-pass fp32
PE mode, 4x the fp32 matmul rate; walrus requires producers to round to it).
"""

import sys
from contextlib import ExitStack

import numpy as np

sys.path.insert(0, "/opt/trn_rl_repo")

import concourse.bass as bass
import concourse.mybir as mybir
import concourse.tile as tile
from concourse import bacc, bass_utils

F32 = mybir.dt.float32
F32R = mybir.dt.float32r

D = 1024          # model dim
T = 2048          # seq len
B = 2             # batch
NH = 16           # heads
HD = 64           # head dim
NCORES = 8
HPC = NH // NCORES  # heads per core = 2
TT = B * T        # total tokens = 4096
EPS = 1e-5
QC = 1024         # attention q-chunk (free dim per softmax tile)
KTILE = 128       # attention k-tile (partition dim)
VSTRIDE = HD + 1  # V_aug column stride (64 v cols + 1 ones col)

MM_DT = F32R      # matmul dtype: float32r = full-rate fp32 PE mode


def build_kernel(tc, xT, wqkv, wout0, wout1, cosT, sinT, ident, onesd, out):
    nc = tc.nc
    Exp = mybir.ActivationFunctionType.Exp
    Ln = mybir.ActivationFunctionType.Ln

    with ExitStack() as ctx:
        # ---------------- persistent tensors ----------------
        const = ctx.enter_context(tc.tile_pool(name="const", bufs=1))
        persist = ctx.enter_context(tc.tile_pool(name="persist", bufs=1))

        ones_sb = const.tile([128, 128], MM_DT)  # all-ones (from DRAM; memset
        nc.sync.dma_start(ones_sb[:], onesd[:])  # cannot write float32r)
        ones_t = ones_sb[:, 0:1]                 # sums-matmul stationary
        ones_bc = ones_sb                        # K=1 broadcast-matmul stationary
        eps_t = const.tile([1, 1], F32)          # bias AP for the Ln activation
        nc.vector.memset(eps_t[:], EPS)
        id_hi = const.tile([128, HD], MM_DT)     # identities for PE transposes:
        nc.sync.dma_start(id_hi[0:HD, :], ident[:])      # rows 0:64 for head 0
        nc.sync.dma_start(id_hi[HD:128, :], ident[:])    # rows 64:128 for head 1

        # weights resident in SBUF
        w_tiles = []                              # 8 x [128, 384] k-chunks of w_qkv'
        for k in range(8):
            wt = const.tile([128, 3 * HPC * HD], MM_DT, tag=f"wq{k}")
            nc.sync.dma_start(wt[:], wqkv[k * 128:(k + 1) * 128, :])
            w_tiles.append(wt)
        w_o0 = const.tile([HD, D], MM_DT, tag="wo0")
        w_o1 = const.tile([HD, D], MM_DT, tag="wo1")
        nc.sync.dma_start(w_o0[:], wout0[:])
        nc.sync.dma_start(w_o1[:], wout1[:])

        QT = persist.tile([128, TT], MM_DT, tag="QT")   # q^T, 2 heads stacked
        KTt = persist.tile([128, TT], MM_DT, tag="KT")  # k^T
        VT = persist.tile([128, TT], MM_DT, tag="VT")   # v^T
        inv_row = persist.tile([1, TT], MM_DT, tag="inv")  # rsqrt(mean sq + eps)
        vaug0 = persist.tile([128, 32 * VSTRIDE], MM_DT, tag="va0")  # V + ones col
        vaug1 = persist.tile([128, 32 * VSTRIDE], MM_DT, tag="va1")
        AO0 = persist.tile([HD, TT], MM_DT, tag="ao0")  # per-head attnout^T
        AO1 = persist.tile([HD, TT], MM_DT, tag="ao1")
        sums0 = persist.tile([65, TT], MM_DT, tag="sm0")  # softmax denom, row 64
        sums1 = persist.tile([65, TT], MM_DT, tag="sm1")

        # ---------------- phase A: stream x^T -> QKV^T + sum-of-squares ----
        with tc.tile_pool(name="xa", bufs=18) as xpool, \
             tc.tile_pool(name="sq", bufs=3) as sqpool, \
             tc.tile_pool(name="ssp", bufs=2, space="PSUM") as ss_ps, \
             tc.tile_pool(name="qkvp", bufs=4, space="PSUM") as qkv_ps:
            for qc in range(8):                   # 512-token chunks of TT
                qsl = slice(qc * 512, (qc + 1) * 512)
                xts = []
                for k in range(8):
                    xt = xpool.tile([128, 512], MM_DT, name=f"xt{k}", tag="xt")
                    nc.sync.dma_start(xt[:], xT[k * 128:(k + 1) * 128, qsl])
                    xts.append(xt)
                # sum over features of x^2 via ones-matmul (contract partitions)
                ssp = ss_ps.tile([1, 512], F32, tag="ss")
                for k in range(8):
                    sq = sqpool.tile([128, 512], MM_DT)
                    nc.vector.tensor_mul(sq[:], xts[k][:], xts[k][:])
                    nc.tensor.matmul(ssp[:], ones_t[:], sq[:],
                                     start=(k == 0), stop=(k == 7))
                # ln(ms + eps) now; the -0.5 exp happens once at the end
                nc.scalar.activation(inv_row[:, qsl], ssp[:], Ln,
                                     bias=eps_t[:], scale=1.0 / D)
                # QKV^T: [384, 512] = w'^T @ x^T chunk
                for c in range(3):
                    qp = qkv_ps.tile([128, 512], F32, tag="qkv")
                    for k in range(8):
                        nc.tensor.matmul(
                            qp[:], w_tiles[k][:, c * 128:(c + 1) * 128],
                            xts[k][:], start=(k == 0), stop=(k == 7))
                    dst = (QT, KTt, VT)[c]
                    nc.scalar.copy(dst[:, qsl], qp[:])

        tc.strict_bb_all_engine_barrier()
        # inv_row = exp(-0.5 * ln(ms + eps)) = rsqrt(ms + eps)
        nc.scalar.activation(inv_row[:], inv_row[:], Exp, scale=-0.5)

        # ---------------- phase B: RoPE + rms scaling (chunked) -------------
        # q' = q*inv*cos + swap(q)*inv*sin_folded ; k' likewise ; v' = v*inv
        with tc.tile_pool(name="rope", bufs=2) as rp, \
             tc.tile_pool(name="ropeb", bufs=2) as rpb, \
             tc.tile_pool(name="bcp", bufs=2, space="PSUM") as bc_ps:
            for ch in range(4):                   # 1024-token chunks
                csl = slice(ch * 1024, (ch + 1) * 1024)
                irb = bc_ps.tile([128, 1024], F32, tag="irb")
                for fh in range(2):
                    nc.tensor.matmul(
                        irb[:, fh * 512:(fh + 1) * 512], ones_bc[0:1, 0:128],
                        inv_row[:, ch * 1024 + fh * 512:ch * 1024 + (fh + 1) * 512],
                        start=True, stop=True)
                cosr = rp.tile([128, 1024], MM_DT, tag="cos")
                sinr = rp.tile([128, 1024], MM_DT, tag="sin")
                nc.sync.dma_start(cosr[:], cosT[:, csl])
                nc.sync.dma_start(sinr[:], sinT[:, csl])
                nc.vector.tensor_mul(cosr[:], cosr[:], irb[:])
                nc.vector.tensor_mul(sinr[:], sinr[:], irb[:])
                qsw = rpb.tile([128, 1024], MM_DT, tag="qsw")
                for src in (QT, KTt):
                    for h in range(HPC):
                        p = h * HD
                        nc.sync.dma_start(qsw[p:p + 32, :], src[p + 32:p + 64, csl])
                        nc.sync.dma_start(qsw[p + 32:p + 64, :], src[p:p + 32, csl])
                    nc.vector.tensor_mul(qsw[:], qsw[:], sinr[:])
                    nc.vector.tensor_mul(src[:, csl], src[:, csl], cosr[:])
                    nc.vector.tensor_add(src[:, csl], src[:, csl], qsw[:])
                nc.vector.tensor_mul(VT[:, csl], VT[:, csl], irb[:])

        tc.strict_bb_all_engine_barrier()
        # ---------------- phase C: V^T -> V_aug (PE transpose) --------------
        for vaug in (vaug0, vaug1):               # ones columns via strided DMA
            vcols = vaug.rearrange("p (t v) -> p t v", v=VSTRIDE)[:, :, HD]
            nc.sync.dma_start(vcols, onesd[:, 0:32])
        with tc.tile_pool(name="vtp", bufs=4, space="PSUM") as vtps:
            for kt in range(32):                  # 128-token tiles over TT
                for h, vaug in ((0, vaug0), (1, vaug1)):
                    vp = vtps.tile([128, HD], MM_DT)
                    nc.tensor.transpose(
                        vp[:], VT[h * HD:(h + 1) * HD, kt * 128:(kt + 1) * 128],
                        id_hi[h * HD:(h + 1) * HD, :])
                    nc.scalar.copy(
                        vaug[:, kt * VSTRIDE:kt * VSTRIDE + HD], vp[:])

        tc.strict_bb_all_engine_barrier()
        # ---------------- phase D: attention --------------------------------
        with tc.tile_pool(name="sps", bufs=1, space="PSUM") as s_ps, \
             tc.tile_pool(name="avps", bufs=1, space="PSUM") as av_ps, \
             tc.tile_pool(name="pt", bufs=4) as ppool:
            for b in range(B):
                for qi in range(T // QC):         # 1024-wide q chunks
                    q0 = b * T + qi * QC
                    avp = [av_ps.tile([65, QC], F32, tag=f"av{h}", name=f"av{h}")
                           for h in range(HPC)]
                    for kt in range(T // KTILE):  # 128-wide k tiles
                        k0 = b * T + kt * KTILE
                        for h, vaug in ((0, vaug0), (1, vaug1)):
                            hs = slice(h * HD, (h + 1) * HD)
                            sp = s_ps.tile([128, QC], F32, tag=f"s{h}")
                            for fh in range(QC // 512):
                                fsl = slice(fh * 512, (fh + 1) * 512)
                                nc.tensor.matmul(
                                    sp[:, fsl], KTt[hs, k0:k0 + KTILE],
                                    QT[hs, q0 + fh * 512:q0 + (fh + 1) * 512],
                                    start=True, stop=True)
                            pt = ppool.tile([128, QC], MM_DT)
                            nc.scalar.activation(pt[:], sp[:], Exp,
                                                 scale=float(HD) ** -0.5)
                            vsl = slice((b * 16 + kt) * VSTRIDE,
                                        (b * 16 + kt) * VSTRIDE + VSTRIDE)
                            for fh in range(QC // 512):
                                fsl = slice(fh * 512, (fh + 1) * 512)
                                nc.tensor.matmul(
                                    avp[h][:, fsl], vaug[:, vsl], pt[:, fsl],
                                    start=(kt == 0), stop=(kt == T // KTILE - 1))
                    for h, (ao, sums) in enumerate(((AO0, sums0), (AO1, sums1))):
                        nc.scalar.copy(ao[:, q0:q0 + QC], avp[h][0:HD, :])
                        nc.scalar.copy(sums[HD:HD + 1, q0:q0 + QC],
                                       avp[h][HD:HD + 1, :])

        tc.strict_bb_all_engine_barrier()
        # ---------------- phase E: 1/denominator, scale attnout --------------
        for sums in (sums0, sums1):
            nc.scalar.activation(sums[HD:HD + 1, :], sums[HD:HD + 1, :], Ln)
            nc.scalar.activation(sums[HD:HD + 1, :], sums[HD:HD + 1, :], Exp,
                                 scale=-1.0)
        with tc.tile_pool(name="rbp", bufs=2, space="PSUM") as rb_ps:
            for ch in range(4):
                csl = slice(ch * 1024, (ch + 1) * 1024)
                for ao, sums in ((AO0, sums0), (AO1, sums1)):
                    rb = rb_ps.tile([HD, 1024], F32, name="rb", tag="rb")
                    for fh in range(2):
                        nc.tensor.matmul(
                            rb[:, fh * 512:(fh + 1) * 512],
                            ones_bc[HD:HD + 1, 0:HD],
                            sums[HD:HD + 1, ch * 1024 + fh * 512:
                                 ch * 1024 + (fh + 1) * 512],
                            start=True, stop=True)
                    nc.vector.tensor_mul(ao[:, csl], ao[:, csl], rb[:])

        tc.strict_bb_all_engine_barrier()
        # ---------------- phase F: out projection ----------------------------
        with tc.tile_pool(name="ops", bufs=4, space="PSUM") as o_ps, \
             tc.tile_pool(name="ot", bufs=4) as opool:
            for tt in range(TT // 128):
                tsl = slice(tt * 128, (tt + 1) * 128)
                for n in range(2):
                    nsl = slice(n * 512, (n + 1) * 512)
                    op = o_ps.tile([128, 512], F32)
                    nc.tensor.matmul(op[:], AO0[:, tsl], w_o0[:, nsl],
                                     start=True, stop=False)
                    nc.tensor.matmul(op[:], AO1[:, tsl], w_o1[:, nsl],
                                     start=False, stop=True)
                    ot = opool.tile([128, 512], F32)
                    nc.vector.tensor_copy(ot[:], op[:])
                    nc.sync.dma_start(out[tsl, nsl], ot[:])


_NC_CACHE = {}


def build_nc():
    key = "main"
    if key in _NC_CACHE:
        return _NC_CACHE[key]
    nc = bacc.Bacc("TRN2", target_bir_lowering=False, debug=False,
                   num_devices=NCORES)
    xT = nc.dram_tensor("xT", [D, TT], MM_DT, kind="ExternalInput").ap()
    wqkv = nc.dram_tensor("wqkv", [D, 3 * HPC * HD], MM_DT,
                          kind="ExternalInput").ap()
    wout0 = nc.dram_tensor("wout0", [HD, D], MM_DT, kind="ExternalInput").ap()
    wout1 = nc.dram_tensor("wout1", [HD, D], MM_DT, kind="ExternalInput").ap()
    cosT = nc.dram_tensor("cosT", [2 * HD, TT], MM_DT, kind="ExternalInput").ap()
    sinT = nc.dram_tensor("sinT", [2 * HD, TT], MM_DT, kind="ExternalInput").ap()
    ident = nc.dram_tensor("ident", [HD, HD], MM_DT, kind="ExternalInput").ap()
    onesd = nc.dram_tensor("onesd", [128, 128], MM_DT, kind="ExternalInput").ap()
    out = nc.dram_tensor("out", [TT, D], F32, kind="ExternalOutput").ap()
    with tile.TileContext(nc) as tc:
        build_kernel(tc, xT, wqkv, wout0, wout1, cosT, sinT, ident, onesd, out)
    nc.compile()
    _NC_CACHE[key] = nc
    return nc


def host_inputs(x, norm_w, w_qkv, w_out):
    """Build the per-core input maps (all host-side numpy prep)."""
    x = np.asarray(x, dtype=np.float32)
    norm_w = np.asarray(norm_w, dtype=np.float32)
    w_qkv = np.asarray(w_qkv, dtype=np.float32)
    w_out = np.asarray(w_out, dtype=np.float32)

    xf = x.reshape(TT, D)
    xT = np.ascontiguousarray(xf.T)

    # rope tables, transposed, tiled over batch, stacked over the 2 local heads
    inv_freq = 1.0 / (10000.0 ** (np.arange(0, HD, 2, dtype=np.float32) / HD))
    t = np.arange(T, dtype=np.float32)
    freqs = np.einsum("i,j->ij", t, inv_freq)
    emb = np.concatenate([freqs, freqs], axis=-1)          # [T, HD]
    cosTb = np.tile(np.cos(emb).T.astype(np.float32), (HPC, B))  # [128, TT]
    sinf = np.sin(emb).T.astype(np.float32)                # [HD, T]
    sinf[:HD // 2] = -sinf[:HD // 2]                       # fold rotate_half sign
    sinTb = np.tile(sinf, (HPC, B))

    wq = (w_qkv * norm_w[:, None]).reshape(D, 3, NH, HD)
    wo = w_out.reshape(NH, HD, D)
    ident = np.eye(HD, dtype=np.float32)

    in_maps = []
    for c in range(NCORES):
        hs = slice(c * HPC, (c + 1) * HPC)
        w_sh = np.ascontiguousarray(wq[:, :, hs, :].reshape(D, 3 * HPC * HD))
        in_maps.append({
            "xT": xT,
            "wqkv": w_sh,
            "wout0": np.ascontiguousarray(wo[c * HPC]),
            "wout1": np.ascontiguousarray(wo[c * HPC + 1]),
            "cosT": cosTb,
            "sinT": sinTb,
            "ident": ident,
            "onesd": np.ones((128, 128), dtype=np.float32),
        })
    return in_maps, xf


def kernel(x, norm_w, w_qkv, w_out, _trace=False):
    nc = build_nc()
    in_maps, xf = host_inputs(x, norm_w, w_qkv, w_out)
    res = bass_utils.run_bass_kernel_spmd(
        nc, in_maps, list(range(NCORES)), trace=_trace)
    acc = xf.copy()
    for r in res.results:
        acc += r["out"]
    kernel.last_results = res
    return acc.reshape(B, T, D)
